# revision 1
# baseline (speedup 1.0000x reference)
"""Trainium2 Bass kernel for nn_Block_39067022524586 (moe_routing).

Strategy (8 NeuronCores):
  Launch 1 — attention phase, data-parallel over tokens: core c owns 2048
    consecutive tokens (batch c//2, sequence half c%2) plus a 64-token left
    halo for the causal grouped conv. Computes
    xa = resid + attn_scale*attn_out and m = rms_norm(xa).
  Host — routes m tokens by expert using sort_idx (known on host).
  Launch 2 — expert MLP, expert-parallel: core c owns expert c and its 2048
    routed tokens. Computes relu(ms @ fc_w)^2 @ proj_w.
  Host — scatters expert outputs back and adds the final residual.

All activations channel-major on-chip ([128, 8, cols], channel = 128*k + p) so
every matmul contracts over the partition dim with stationary weights.
Matmul dtypes: bf16 on the attention branch (its contribution to the output is
~1e-5 relative — measured), float32r (full-speed, ~1.7e-4) for the MLP and all
rms-norm statistics. Residual/norm arithmetic is exact fp32.
"""
import sys

for _p in ("/opt/trn_rl_repo", "/root/.axon_site/_ro/trn_rl_repo"):
    if _p not in sys.path:
        sys.path.insert(0, _p)

import numpy as np
import ml_dtypes

import concourse.bass as bass
import concourse.mybir as mybir
import concourse.tile as tile

F32 = mybir.dt.float32
F32R = mybir.dt.float32r
BF16 = mybir.dt.bfloat16
AF = mybir.ActivationFunctionType
EPS = 1.1920929e-07
HALO, EXT, T, NT = 64, 2112, 2048, 512

# ---------------------------------------------------------------------------
# Compiler workarounds: this walrus build accepts at most one sync wait per
# instruction, and the InstDrain codegen path accepts none.
# ---------------------------------------------------------------------------
_patch_state = {"applied": False}


def _apply_patches():
    if _patch_state["applied"]:
        return
    _patch_state["applied"] = True
    import bass_rust
    from concourse.tile import ScopedClock

    def _patched_drain_and_barrier(self, tick_clock, wait_clock):
        nc = self.nc
        drain_inst = nc.sync.drain()
        wait_clock.add_sem_waits(drain_inst.ins,
                                 ScopedClock({None: tick_clock.global_clock}))
        si = drain_inst.ins.sync_info
        waits = list(si.on_wait) if si is not None else []
        if waits:
            si.on_wait = []
            for w in waits:
                n = nc.sync.nop()
                n.ins.sync_info = bass_rust.SyncInfo(on_wait=[w], on_update=[])
        nc.all_engine_barrier()
        assert self.sems is not None
        popped = nc._tile_sem_poison_stack.pop()
        assert popped is self._sem_poison
        nc.clear_and_free_semaphores(list(self.sems.allocated().values()))
        nc.all_engine_barrier()

    tile.TileContext._drain_and_barrier = _patched_drain_and_barrier

    _ctr = [0]

    def _split_multiwait_bir(bir_json):
        import orjson
        j = orjson.loads(bir_json)
        changed = False
        for fn in j.get("functions", []):
            for bb in fn.get("blocks", []):
                out = []
                for inst in bb.get("instructions", []):
                    si = inst.get("sync_info")
                    ow = (si or {}).get("on_wait") or []
                    if len(ow) > 1:
                        changed = True
                        for w in ow[:-1]:
                            _ctr[0] += 1
                            out.append({
                                "debug": inst.get("debug", 0),
                                "engine": inst["engine"],
                                "ins": [], "outs": [],
                                "name": f"I-mwfix-{_ctr[0]}",
                                "opcode": "EventSemaphore",
                                "sync_info": {"on_update": [], "on_wait": [w]},
                            })
                        si["on_wait"] = [ow[-1]]
                    out.append(inst)
                bb["instructions"] = out
        return orjson.dumps(j) if changed else bir_json

    from concourse import bass_utils, bass2jax
    orig_compile = bass_utils.compile_bir_kernel

    def patched_compile(bir_json, tmpdir, neff_name="file.neff"):
        return orig_compile(_split_multiwait_bir(bytes(bir_json)), tmpdir, neff_name)

    bass_utils.compile_bir_kernel = patched_compile
    bass2jax.compile_bir_kernel = patched_compile


# ---------------------------------------------------------------------------
# Launch 1: attention phase
# ---------------------------------------------------------------------------
def build_attn_nc(rep=1):
    nc = bass.Bass()
    xT = nc.dram_tensor("xT", [128, 8, EXT], F32, kind="ExternalInput")
    x0T = nc.dram_tensor("x0T", [128, 8, EXT], F32, kind="ExternalInput")
    qw = nc.dram_tensor("qw", [128, 8, 8, 128], BF16, kind="ExternalInput")
    ow = nc.dram_tensor("ow", [128, 8, 8, 128], BF16, kind="ExternalInput")
    cw = nc.dram_tensor("cw", [128, 16, 32, 64], BF16, kind="ExternalInput")
    hmask = nc.dram_tensor("hmask", [128, 8, 16], BF16, kind="ExternalInput")
    bmask = nc.dram_tensor("bmask", [16, 8, 128], BF16, kind="ExternalInput")
    muc = nc.dram_tensor("muc", [128, 8], F32, kind="ExternalInput")
    negg = nc.dram_tensor("negg", [16, 1], F32, kind="ExternalInput")
    rm0 = nc.dram_tensor("rm0", [128, 8], F32, kind="ExternalInput")
    rm1 = nc.dram_tensor("rm1", [128, 8], F32, kind="ExternalInput")
    asc = nc.dram_tensor("asc", [128, 8], F32, kind="ExternalInput")
    onesr = nc.dram_tensor("onesr", [128, 128], F32R, kind="ExternalInput")
    epsb = nc.dram_tensor("epsb", [128, 1], F32, kind="ExternalInput")
    xaT = nc.dram_tensor("xaT", [128, 8, T], F32, kind="ExternalOutput")
    mT = nc.dram_tensor("mT", [128, 8, T], F32, kind="ExternalOutput")
    nsc = nc.dram_tensor("nsc", [128, 8, EXT], BF16)
    xrsc = nc.dram_tensor("xrsc", [128, 8, EXT], F32)

    with tile.TileContext(nc) as tc:
        with (
            tc.tile_pool(name="res", bufs=1) as res,
            tc.tile_pool(name="wk", bufs=2) as wk,
            tc.tile_pool(name="ps", bufs=1, space="PSUM") as psp,
        ):
            qw_s = res.tile([128, 8, 8, 128], BF16, tag="qw")
            ow_s = res.tile([128, 8, 8, 128], BF16, tag="ow")
            cw_s = res.tile([128, 16, 32, 64], BF16, tag="cw")
            hm_s = res.tile([128, 8, 16], BF16, tag="hm")
            bm_s = res.tile([16, 8, 128], BF16, tag="bm")
            muc_s = res.tile([128, 8], F32, tag="muc")
            negg_s = res.tile([16, 1], F32, tag="negg")
            rm0_s = res.tile([128, 8], F32, tag="rm0")
            rm1_s = res.tile([128, 8], F32, tag="rm1")
            asc_s = res.tile([128, 8], F32, tag="asc")
            ones_s = res.tile([128, 128], F32R, tag="onesr")
            eps_s = res.tile([128, 1], F32, tag="epsb")
            for dst, src in [(qw_s, qw), (ow_s, ow), (cw_s, cw), (hm_s, hmask),
                             (bm_s, bmask), (muc_s, muc), (negg_s, negg),
                             (rm0_s, rm0), (rm1_s, rm1), (asc_s, asc),
                             (ones_s, onesr), (eps_s, epsb)]:
                nc.sync.dma_start(dst[:], src[:])

            for _ in range(rep):
                # Phase A: xr = resid mix, n = rms_norm(xr) -> DRAM scratch
                offs = [(i * NT, NT) for i in range(4)] + [(4 * NT, EXT - 4 * NT)]
                for off, W in offs:
                    xr8 = wk.tile([128, 8, NT], F32, tag="big8f32", bufs=1)
                    ps_ss = psp.tile([128, NT], F32, tag="ss", bufs=1)
                    for d in range(8):
                        xd = wk.tile([128, NT], F32, tag="xd")
                        x0d = wk.tile([128, NT], F32, tag="x0d")
                        nc.sync.dma_start(xd[:, :W], xT[:, d, off:off + W])
                        nc.sync.dma_start(x0d[:, :W], x0T[:, d, off:off + W])
                        tt = wk.tile([128, NT], F32, tag="tt")
                        nc.vector.tensor_scalar_mul(tt[:, :W], x0d[:, :W],
                                                    rm1_s[:, d:d + 1])
                        nc.vector.scalar_tensor_tensor(
                            xr8[:, d, :W], xd[:, :W], rm0_s[:, d:d + 1], tt[:, :W],
                            mybir.AluOpType.mult, mybir.AluOpType.add)
                        nc.sync.dma_start(xrsc[:, d, off:off + W], xr8[:, d, :W])
                        sq = wk.tile([128, NT], F32R, tag="sq")
                        nc.vector.tensor_mul(sq[:, :W], xr8[:, d, :W], xr8[:, d, :W])
                        nc.tensor.matmul(ps_ss[:, :W], ones_s[:], sq[:, :W],
                                         start=(d == 0), stop=(d == 7))
                    srt = wk.tile([128, NT], F32, tag="srt")
                    nc.scalar.activation(srt[:, :W], ps_ss[:, :W], AF.Sqrt,
                                         bias=eps_s[:, 0:1], scale=1.0 / 1024.0)
                    rs = wk.tile([128, NT], F32, tag="rs")
                    nc.vector.reciprocal(rs[:, :W], srt[:, :W])
                    for d in range(8):
                        nd = wk.tile([128, NT], BF16, tag="nd")
                        nc.vector.tensor_mul(nd[:, :W], xr8[:, d, :W], rs[:, :W])
                        nc.sync.dma_start(nsc[:, d, off:off + W], nd[:, :W])

                # Phase B: attention per 512-token tile
                for t in range(4):
                    c0 = HALO + NT * t
                    nqh = wk.tile([128, 8, NT + 64], BF16, tag="nqh", bufs=2)
                    nc.sync.dma_start(nqh[:], nsc[:, :, c0 - 64:c0 + NT])
                    ps_e = psp.tile([16, NT], F32, tag="e", bufs=1)
                    for d in range(8):
                        nm = wk.tile([128, NT], BF16, tag="nm")
                        nc.vector.tensor_scalar_sub(nm[:], nqh[:, d, 64:],
                                                    muc_s[:, d:d + 1])
                        sqe = wk.tile([128, NT], BF16, tag="sqe")
                        nc.vector.tensor_mul(sqe[:], nm[:], nm[:])
                        nc.tensor.matmul(ps_e[:], hm_s[:, d, :], sqe[:],
                                         start=(d == 0), stop=(d == 7))
                    ebuf = wk.tile([16, NT], F32, tag="ebuf")
                    nc.scalar.activation(ebuf[:], ps_e[:], AF.Sqrt)
                    g16 = wk.tile([16, NT], BF16, tag="g16")
                    nc.scalar.activation(g16[:], ebuf[:], AF.Sigmoid,
                                         scale=negg_s[:, 0:1])
                    ao8 = wk.tile([128, 8, NT], BF16, tag="ao8", bufs=1)
                    for d in range(8):
                        ps_q = psp.tile([128, NT], F32, tag="q", bufs=2)
                        for k in range(8):
                            nc.tensor.matmul(ps_q[:], qw_s[:, k, d, :],
                                             nqh[:, k, 64:],
                                             start=(k == 0), stop=(k == 7))
                        qd = wk.tile([128, NT], BF16, tag="qd")
                        nc.scalar.activation(qd[:], ps_q[:], AF.Sigmoid)
                        ps_c = psp.tile([128, NT], F32, tag="c", bufs=2)
                        for gg in range(2):
                            g = 2 * d + gg
                            cxx = wk.tile([128, NT + 64], BF16, tag="cxx", bufs=3)
                            nc.sync.dma_start(
                                cxx[0:64, 0:NT + 62],
                                nsc[64 * gg:64 * gg + 64, d, c0 - 63:c0 + NT - 1])
                            nc.sync.dma_start(
                                cxx[64:128, 0:NT + 62],
                                nsc[64 * gg:64 * gg + 64, d, c0 - 62:c0 + NT])
                            for kb in range(32):
                                nc.tensor.matmul(
                                    ps_c[64 * gg:64 * gg + 64, :],
                                    cw_s[:, g, kb, :], cxx[:, 2 * kb:2 * kb + NT],
                                    tile_position=(0, 64 * gg),
                                    start=(kb == 0), stop=(kb == 31),
                                    skip_group_check=True)
                        ps_g = psp.tile([128, NT], F32, tag="g", bufs=1)
                        nc.tensor.matmul(ps_g[:], bm_s[:, d, :], g16[:],
                                         start=True, stop=True)
                        t1 = wk.tile([128, NT], BF16, tag="t1")
                        nc.vector.tensor_mul(t1[:], qd[:], ps_c[:])
                        nc.vector.tensor_mul(ao8[:, d, :], t1[:], ps_g[:])
                    xa8 = wk.tile([128, 8, NT], F32, tag="big8f32", bufs=1)
                    ps_s2 = psp.tile([128, NT], F32, tag="ss", bufs=1)
                    for do in range(8):
                        ps_o = psp.tile([128, NT], F32, tag="o", bufs=1)
                        for k in range(8):
                            nc.tensor.matmul(ps_o[:], ow_s[:, k, do, :],
                                             ao8[:, k, :],
                                             start=(k == 0), stop=(k == 7))
                        xrd = wk.tile([128, NT], F32, tag="xrd")
                        nc.sync.dma_start(xrd[:], xrsc[:, do, c0:c0 + NT])
                        nc.vector.scalar_tensor_tensor(
                            xa8[:, do, :], ps_o[:], asc_s[:, do:do + 1], xrd[:],
                            mybir.AluOpType.mult, mybir.AluOpType.add)
                        nc.sync.dma_start(xaT[:, do, NT * t:NT * t + NT],
                                          xa8[:, do, :])
                        sq2 = wk.tile([128, NT], F32R, tag="sq")
                        nc.vector.tensor_mul(sq2[:], xa8[:, do, :], xa8[:, do, :])
                        nc.tensor.matmul(ps_s2[:], ones_s[:], sq2[:],
                                         start=(do == 0), stop=(do == 7))
                    srt2 = wk.tile([128, NT], F32, tag="srt")
                    nc.scalar.activation(srt2[:], ps_s2[:], AF.Sqrt,
                                         bias=eps_s[:, 0:1], scale=1.0 / 1024.0)
                    rs2 = wk.tile([128, NT], F32, tag="rs")
                    nc.vector.reciprocal(rs2[:], srt2[:])
                    for d in range(8):
                        md = wk.tile([128, NT], F32, tag="md")
                        nc.vector.tensor_mul(md[:], xa8[:, d, :], rs2[:])
                        nc.sync.dma_start(mT[:, d, NT * t:NT * t + NT], md[:])
    return nc


# ---------------------------------------------------------------------------
# Launch 2: expert MLP
# ---------------------------------------------------------------------------
def build_mlp_nc(rep=1):
    nc = bass.Bass()
    ms = nc.dram_tensor("msT", [128, 8, T], F32R, kind="ExternalInput")
    fcw = nc.dram_tensor("fcw", [128, 8, 4, 128], F32R, kind="ExternalInput")
    pjw = nc.dram_tensor("pjw", [128, 4, 8, 128], F32R, kind="ExternalInput")
    y = nc.dram_tensor("yT", [128, 8, T], F32, kind="ExternalOutput")

    with tile.TileContext(nc) as tc:
        with (
            tc.tile_pool(name="wp", bufs=1) as wp,
            tc.tile_pool(name="act", bufs=2) as act,
            tc.tile_pool(name="ps", bufs=1, space="PSUM") as psp,
        ):
            fcw_s = wp.tile([128, 8, 4, 128], F32R, tag="fcw")
            pjw_s = wp.tile([128, 4, 8, 128], F32R, tag="pjw")
            ms_s = wp.tile([128, 8, T], F32R, tag="ms")
            y_s = wp.tile([128, 8, T], F32, tag="y")
            nc.sync.dma_start(fcw_s[:], fcw[:])
            nc.sync.dma_start(pjw_s[:], pjw[:])
            nc.sync.dma_start(ms_s[:], ms[:])

            for _ in range(rep):
                for t in range(T // NT):
                    sl = slice(t * NT, (t + 1) * NT)
                    h2 = act.tile([128, 4, NT], F32R, tag="h2")
                    for mi in range(4):
                        ph = psp.tile([128, NT], F32, tag="ph", bufs=2)
                        for k in range(8):
                            nc.tensor.matmul(ph[:], fcw_s[:, k, mi, :],
                                             ms_s[:, k, sl],
                                             start=(k == 0), stop=(k == 7))
                        r = act.tile([128, NT], F32R, tag="r")
                        nc.scalar.activation(r[:], ph[:], AF.Relu)
                        nc.vector.tensor_mul(h2[:, mi, :], r[:], r[:])
                    for do in range(8):
                        py = psp.tile([128, NT], F32, tag="py", bufs=2)
                        for ki in range(4):
                            nc.tensor.matmul(py[:], pjw_s[:, ki, do, :],
                                             h2[:, ki, :],
                                             start=(ki == 0), stop=(ki == 3))
                        nc.scalar.activation(y_s[:, do, sl], py[:], AF.Copy)
            nc.sync.dma_start(y[:], y_s[:])
    return nc


# ---------------------------------------------------------------------------
# Host-side packing
# ---------------------------------------------------------------------------
def tile_chanmajor(a_T):
    return np.ascontiguousarray(a_T.reshape(8, 128, -1).transpose(1, 0, 2))


def untile_chanmajor(a):
    return np.ascontiguousarray(a.transpose(1, 0, 2)).reshape(1024, -1)


def pack_proj_T(w):
    return np.ascontiguousarray(w.T.reshape(8, 128, 8, 128).transpose(1, 0, 2, 3))


def pack_conv(conv_w):
    tmp = conv_w.reshape(16, 64, 64, 32, 2)
    return np.ascontiguousarray(
        tmp.transpose(4, 2, 0, 3, 1).reshape(128, 16, 32, 64))


def pack_vec(v):
    return np.ascontiguousarray(v.reshape(8, 128).T)


def pack_fcw(fc_w_e):
    return np.ascontiguousarray(
        fc_w_e.reshape(8, 128, 4, 128).transpose(1, 0, 2, 3))


def pack_pjw(proj_w_e):
    return np.ascontiguousarray(
        proj_w_e.reshape(4, 128, 8, 128).transpose(1, 0, 2, 3))


def make_masks():
    head = np.arange(1024) // 64
    ch = head.reshape(8, 128)
    hm = np.zeros((128, 8, 16), np.float32)
    bm = np.zeros((16, 8, 128), np.float32)
    for dd in range(8):
        for p in range(128):
            hm[p, dd, ch[dd, p]] = 1.0
            bm[ch[dd, p], dd, p] = 1.0
    return hm, bm


_CACHE = {}


def _get_ncs():
    if "attn" not in _CACHE:
        _apply_patches()
        _CACHE["attn"] = build_attn_nc()
        _CACHE["mlp"] = build_mlp_nc()
    return _CACHE["attn"], _CACHE["mlp"]


def kernel(x, x0, mu, beta, q_proj_w, conv_w, out_proj_w, fc_w, proj_w,
           attn_scale, mlp_scale, resid_mix, sort_idx):
    from concourse.bass_utils import run_bass_kernel_spmd

    nc1, nc2 = _get_ncs()
    f32 = np.float32
    x = np.asarray(x, f32)
    x0 = np.asarray(x0, f32)
    mu = np.asarray(mu, f32)
    beta = np.asarray(beta, f32)
    q_proj_w = np.asarray(q_proj_w, f32)
    conv_w = np.asarray(conv_w, f32)
    out_proj_w = np.asarray(out_proj_w, f32)
    fc_w = np.asarray(fc_w, f32)
    proj_w = np.asarray(proj_w, f32)
    attn_scale = np.asarray(attn_scale, f32)
    mlp_scale = np.asarray(mlp_scale, f32)
    resid_mix = np.asarray(resid_mix, f32)
    idx = np.asarray(sort_idx).astype(np.int64)

    hm, bm = make_masks()
    common = {
        "qw": pack_proj_T(q_proj_w).astype(ml_dtypes.bfloat16),
        "ow": pack_proj_T(out_proj_w).astype(ml_dtypes.bfloat16),
        "cw": pack_conv(conv_w).astype(ml_dtypes.bfloat16),
        "hmask": hm.astype(ml_dtypes.bfloat16),
        "bmask": bm.astype(ml_dtypes.bfloat16),
        "muc": pack_vec(mu.reshape(-1)),
        "negg": (-np.log1p(np.exp(beta))).reshape(16, 1).astype(f32),
        "rm0": pack_vec(resid_mix[0]),
        "rm1": pack_vec(resid_mix[1]),
        "asc": pack_vec(attn_scale),
        "onesr": np.ones((128, 128), f32),
        "epsb": np.full((128, 1), EPS, f32),
    }
    in_maps1 = []
    for c in range(8):
        b, h = c // 2, c % 2
        s0 = h * 2048
        ext = np.zeros((1024, EXT), f32)
        ext0 = np.zeros((1024, EXT), f32)
        ext[:, HALO:] = x[b, s0:s0 + T].T
        ext0[:, HALO:] = x0[b, s0:s0 + T].T
        if s0 > 0:
            ext[:, :HALO] = x[b, s0 - HALO:s0].T
            ext0[:, :HALO] = x0[b, s0 - HALO:s0].T
        in_maps1.append({"xT": tile_chanmajor(ext), "x0T": tile_chanmajor(ext0),
                         **common})

    res1 = run_bass_kernel_spmd(nc1, in_maps1, core_ids=list(range(8)))

    # host routing: token-major xa, expert-sorted m
    xa_tok = np.concatenate(
        [untile_chanmajor(res1.results[c]["xaT"]).T for c in range(8)], axis=0)
    m_chan = np.concatenate(
        [untile_chanmajor(res1.results[c]["mT"]) for c in range(8)], axis=1)
    ms_all = m_chan[:, idx]                          # [1024, 16384] sorted

    in_maps2 = []
    for c in range(8):
        in_maps2.append({
            "msT": tile_chanmajor(ms_all[:, c * T:(c + 1) * T]),
            "fcw": pack_fcw(fc_w[c]),
            "pjw": pack_pjw(proj_w[c]),
        })
    res2 = run_bass_kernel_spmd(nc2, in_maps2, core_ids=list(range(8)))

    y_sorted_tok = np.concatenate(
        [untile_chanmajor(res2.results[c]["yT"]).T for c in range(8)], axis=0)

    out = xa_tok
    out[idx] += mlp_scale[None, :] * y_sorted_tok
    return np.ascontiguousarray(out.reshape(4, 4096, 1024), dtype=f32)


# revision 2
# speedup vs baseline: 1.1741x; 1.1741x over previous
"""Trainium2 Bass kernel for nn_Block_39067022524586 (moe_routing).

Strategy (8 NeuronCores):
  Launch 1 — attention phase, data-parallel over tokens: core c owns 2048
    consecutive tokens (batch c//2, sequence half c%2) plus a 64-token left
    halo for the causal grouped conv. Computes
    xa = resid + attn_scale*attn_out and m = rms_norm(xa).
  Host — routes m tokens by expert using sort_idx (known on host).
  Launch 2 — expert MLP, expert-parallel: core c owns expert c and its 2048
    routed tokens. Computes relu(ms @ fc_w)^2 @ proj_w.
  Host — scatters expert outputs back and adds the final residual.

All activations channel-major on-chip ([128, 8, cols], channel = 128*k + p) so
every matmul contracts over the partition dim with stationary weights.
Matmul dtypes: bf16 on the attention branch (its contribution to the output is
~1e-5 relative — measured), float32r (full-speed, ~1.7e-4) for the MLP and all
rms-norm statistics. Residual/norm arithmetic is exact fp32.
"""
import sys

for _p in ("/opt/trn_rl_repo", "/root/.axon_site/_ro/trn_rl_repo"):
    if _p not in sys.path:
        sys.path.insert(0, _p)

import numpy as np
import ml_dtypes

import concourse.bass as bass
import concourse.mybir as mybir
import concourse.tile as tile

F32 = mybir.dt.float32
F32R = mybir.dt.float32r
BF16 = mybir.dt.bfloat16
F16 = mybir.dt.float16
AF = mybir.ActivationFunctionType
EPS = 1.1920929e-07
HALO, EXT, T, NT = 64, 2112, 2048, 512

# ---------------------------------------------------------------------------
# Compiler workarounds: this walrus build accepts at most one sync wait per
# instruction, and the InstDrain codegen path accepts none.
# ---------------------------------------------------------------------------
_patch_state = {"applied": False}


def _apply_patches():
    if _patch_state["applied"]:
        return
    _patch_state["applied"] = True
    import bass_rust
    from concourse.tile import ScopedClock

    def _patched_drain_and_barrier(self, tick_clock, wait_clock):
        nc = self.nc
        drain_inst = nc.sync.drain()
        wait_clock.add_sem_waits(drain_inst.ins,
                                 ScopedClock({None: tick_clock.global_clock}))
        si = drain_inst.ins.sync_info
        waits = list(si.on_wait) if si is not None else []
        if waits:
            si.on_wait = []
            for w in waits:
                n = nc.sync.nop()
                n.ins.sync_info = bass_rust.SyncInfo(on_wait=[w], on_update=[])
        nc.all_engine_barrier()
        assert self.sems is not None
        popped = nc._tile_sem_poison_stack.pop()
        assert popped is self._sem_poison
        nc.clear_and_free_semaphores(list(self.sems.allocated().values()))
        nc.all_engine_barrier()

    tile.TileContext._drain_and_barrier = _patched_drain_and_barrier

    _ctr = [0]

    def _split_multiwait_bir(bir_json):
        import orjson
        j = orjson.loads(bir_json)
        changed = False
        for fn in j.get("functions", []):
            for bb in fn.get("blocks", []):
                out = []
                for inst in bb.get("instructions", []):
                    si = inst.get("sync_info")
                    ow = (si or {}).get("on_wait") or []
                    if len(ow) > 1:
                        changed = True
                        for w in ow[:-1]:
                            _ctr[0] += 1
                            out.append({
                                "debug": inst.get("debug", 0),
                                "engine": inst["engine"],
                                "ins": [], "outs": [],
                                "name": f"I-mwfix-{_ctr[0]}",
                                "opcode": "EventSemaphore",
                                "sync_info": {"on_update": [], "on_wait": [w]},
                            })
                        si["on_wait"] = [ow[-1]]
                    out.append(inst)
                bb["instructions"] = out
        return orjson.dumps(j) if changed else bir_json

    from concourse import bass_utils, bass2jax
    orig_compile = bass_utils.compile_bir_kernel

    def patched_compile(bir_json, tmpdir, neff_name="file.neff"):
        return orig_compile(_split_multiwait_bir(bytes(bir_json)), tmpdir, neff_name)

    bass_utils.compile_bir_kernel = patched_compile
    bass2jax.compile_bir_kernel = patched_compile


# ---------------------------------------------------------------------------
# Launch 1: attention phase
# ---------------------------------------------------------------------------
def build_attn_nc(rep=1):
    nc = bass.Bass()
    xT = nc.dram_tensor("xT", [128, 8, EXT], F32, kind="ExternalInput")
    x0T = nc.dram_tensor("x0T", [128, 8, EXT], F32, kind="ExternalInput")
    qw = nc.dram_tensor("qw", [128, 8, 8, 128], BF16, kind="ExternalInput")
    ow = nc.dram_tensor("ow", [128, 8, 8, 128], BF16, kind="ExternalInput")
    cw = nc.dram_tensor("cw", [128, 16, 32, 64], BF16, kind="ExternalInput")
    hmask = nc.dram_tensor("hmask", [128, 8, 16], BF16, kind="ExternalInput")
    bmask = nc.dram_tensor("bmask", [16, 8, 128], BF16, kind="ExternalInput")
    muc = nc.dram_tensor("muc", [128, 8], F32, kind="ExternalInput")
    negg = nc.dram_tensor("negg", [16, 1], F32, kind="ExternalInput")
    rm0 = nc.dram_tensor("rm0", [128, 8], F32, kind="ExternalInput")
    rm1 = nc.dram_tensor("rm1", [128, 8], F32, kind="ExternalInput")
    asc = nc.dram_tensor("asc", [128, 8], F32, kind="ExternalInput")
    onesr = nc.dram_tensor("onesr", [128, 128], F16, kind="ExternalInput")
    epsb = nc.dram_tensor("epsb", [128, 1], F32, kind="ExternalInput")
    xaT = nc.dram_tensor("xaT", [128, 8, T], F32, kind="ExternalOutput")
    mT = nc.dram_tensor("mT", [128, 8, T], F32, kind="ExternalOutput")
    nsc = nc.dram_tensor("nsc", [128, 8, EXT], BF16)
    xrsc = nc.dram_tensor("xrsc", [128, 8, EXT], F32)

    with tile.TileContext(nc) as tc:
        with (
            tc.tile_pool(name="res", bufs=1) as res,
            tc.tile_pool(name="wk", bufs=2) as wk,
            tc.tile_pool(name="ps", bufs=1, space="PSUM") as psp,
        ):
            qw_s = res.tile([128, 8, 8, 128], BF16, tag="qw")
            ow_s = res.tile([128, 8, 8, 128], BF16, tag="ow")
            cw_s = res.tile([128, 16, 32, 64], BF16, tag="cw")
            hm_s = res.tile([128, 8, 16], BF16, tag="hm")
            bm_s = res.tile([16, 8, 128], BF16, tag="bm")
            muc_s = res.tile([128, 8], F32, tag="muc")
            negg_s = res.tile([16, 1], F32, tag="negg")
            rm0_s = res.tile([128, 8], F32, tag="rm0")
            rm1_s = res.tile([128, 8], F32, tag="rm1")
            asc_s = res.tile([128, 8], F32, tag="asc")
            ones_s = res.tile([128, 128], F16, tag="onesr")
            eps_s = res.tile([128, 1], F32, tag="epsb")
            for dst, src in [(qw_s, qw), (ow_s, ow), (cw_s, cw), (hm_s, hmask),
                             (bm_s, bmask), (muc_s, muc), (negg_s, negg),
                             (rm0_s, rm0), (rm1_s, rm1), (asc_s, asc),
                             (ones_s, onesr), (eps_s, epsb)]:
                nc.sync.dma_start(dst[:], src[:])

            for _ in range(rep):
                # Phase A: xr = resid mix, n = rms_norm(xr) -> DRAM scratch
                offs = [(i * NT, NT) for i in range(4)] + [(4 * NT, EXT - 4 * NT)]
                for off, W in offs:
                    xr8 = wk.tile([128, 8, NT], F32, tag="big8f32", bufs=1)
                    ps_ss = psp.tile([128, NT], F32, tag="ss", bufs=1)
                    for d in range(8):
                        xd = wk.tile([128, NT], F32, tag="xd")
                        x0d = wk.tile([128, NT], F32, tag="x0d")
                        nc.sync.dma_start(xd[:, :W], xT[:, d, off:off + W])
                        nc.sync.dma_start(x0d[:, :W], x0T[:, d, off:off + W])
                        tt = wk.tile([128, NT], F32, tag="tt")
                        nc.vector.tensor_scalar_mul(tt[:, :W], x0d[:, :W],
                                                    rm1_s[:, d:d + 1])
                        nc.vector.scalar_tensor_tensor(
                            xr8[:, d, :W], xd[:, :W], rm0_s[:, d:d + 1], tt[:, :W],
                            mybir.AluOpType.mult, mybir.AluOpType.add)
                        nc.sync.dma_start(xrsc[:, d, off:off + W], xr8[:, d, :W])
                        sq = wk.tile([128, NT], F16, tag="sq")
                        nc.vector.tensor_mul(sq[:, :W], xr8[:, d, :W], xr8[:, d, :W])
                        nc.tensor.matmul(ps_ss[:, :W], ones_s[:], sq[:, :W],
                                         start=(d == 0), stop=(d == 7))
                    srt = wk.tile([128, NT], F32, tag="srt")
                    nc.scalar.activation(srt[:, :W], ps_ss[:, :W], AF.Sqrt,
                                         bias=eps_s[:, 0:1], scale=1.0 / 1024.0)
                    rs = wk.tile([128, NT], F32, tag="rs")
                    nc.vector.reciprocal(rs[:, :W], srt[:, :W])
                    for d in range(8):
                        nd = wk.tile([128, NT], BF16, tag="nd")
                        nc.vector.tensor_mul(nd[:, :W], xr8[:, d, :W], rs[:, :W])
                        nc.sync.dma_start(nsc[:, d, off:off + W], nd[:, :W])

                # Phase B: attention per 512-token tile
                for t in range(4):
                    c0 = HALO + NT * t
                    nqh = wk.tile([128, 8, NT + 64], BF16, tag="nqh", bufs=2)
                    nc.sync.dma_start(nqh[:], nsc[:, :, c0 - 64:c0 + NT])
                    ps_e = psp.tile([16, NT], F32, tag="e", bufs=1)
                    for d in range(8):
                        nm = wk.tile([128, NT], BF16, tag="nm")
                        nc.vector.tensor_scalar_sub(nm[:], nqh[:, d, 64:],
                                                    muc_s[:, d:d + 1])
                        sqe = wk.tile([128, NT], BF16, tag="sqe")
                        nc.vector.tensor_mul(sqe[:], nm[:], nm[:])
                        nc.tensor.matmul(ps_e[:], hm_s[:, d, :], sqe[:],
                                         start=(d == 0), stop=(d == 7))
                    ebuf = wk.tile([16, NT], F32, tag="ebuf")
                    nc.scalar.activation(ebuf[:], ps_e[:], AF.Sqrt)
                    g16 = wk.tile([16, NT], BF16, tag="g16")
                    nc.scalar.activation(g16[:], ebuf[:], AF.Sigmoid,
                                         scale=negg_s[:, 0:1])
                    ao8 = wk.tile([128, 8, NT], BF16, tag="ao8", bufs=1)
                    for d in range(8):
                        ps_q = psp.tile([128, NT], F32, tag="q", bufs=2)
                        for k in range(8):
                            nc.tensor.matmul(ps_q[:], qw_s[:, k, d, :],
                                             nqh[:, k, 64:],
                                             start=(k == 0), stop=(k == 7))
                        qd = wk.tile([128, NT], BF16, tag="qd")
                        nc.scalar.activation(qd[:], ps_q[:], AF.Sigmoid)
                        ps_c0 = psp.tile([128, NT], F32, tag="c0", bufs=1)
                        ps_c1 = psp.tile([128, NT], F32, tag="c1", bufs=1)
                        for gg in range(2):
                            g = 2 * d + gg
                            psc = ps_c0 if gg == 0 else ps_c1
                            cxx = wk.tile([128, NT + 64], BF16, tag="cxx", bufs=3)
                            nc.sync.dma_start(
                                cxx[0:64, 0:NT + 62],
                                nsc[64 * gg:64 * gg + 64, d, c0 - 63:c0 + NT - 1])
                            nc.sync.dma_start(
                                cxx[64:128, 0:NT + 62],
                                nsc[64 * gg:64 * gg + 64, d, c0 - 62:c0 + NT])
                            for kb in range(32):
                                nc.tensor.matmul(
                                    psc[64 * gg:64 * gg + 64, :],
                                    cw_s[:, g, kb, :], cxx[:, 2 * kb:2 * kb + NT],
                                    tile_position=(0, 64 * gg),
                                    start=(kb == 0), stop=(kb == 31))
                        ps_g = psp.tile([128, NT], F32, tag="g", bufs=1)
                        nc.tensor.matmul(ps_g[:], bm_s[:, d, :], g16[:],
                                         start=True, stop=True)
                        t1 = wk.tile([128, NT], BF16, tag="t1")
                        nc.vector.tensor_mul(t1[0:64, :], qd[0:64, :], ps_c0[0:64, :])
                        nc.vector.tensor_mul(t1[64:128, :], qd[64:128, :], ps_c1[64:128, :])
                        nc.vector.tensor_mul(ao8[:, d, :], t1[:], ps_g[:])
                    xa8 = wk.tile([128, 8, NT], F32, tag="big8f32", bufs=1)
                    ps_s2 = psp.tile([128, NT], F32, tag="ss", bufs=1)
                    for do in range(8):
                        ps_o = psp.tile([128, NT], F32, tag="o", bufs=1)
                        for k in range(8):
                            nc.tensor.matmul(ps_o[:], ow_s[:, k, do, :],
                                             ao8[:, k, :],
                                             start=(k == 0), stop=(k == 7))
                        xrd = wk.tile([128, NT], F32, tag="xrd")
                        nc.sync.dma_start(xrd[:], xrsc[:, do, c0:c0 + NT])
                        nc.vector.scalar_tensor_tensor(
                            xa8[:, do, :], ps_o[:], asc_s[:, do:do + 1], xrd[:],
                            mybir.AluOpType.mult, mybir.AluOpType.add)
                        nc.sync.dma_start(xaT[:, do, NT * t:NT * t + NT],
                                          xa8[:, do, :])
                        sq2 = wk.tile([128, NT], F16, tag="sq")
                        nc.vector.tensor_mul(sq2[:], xa8[:, do, :], xa8[:, do, :])
                        nc.tensor.matmul(ps_s2[:], ones_s[:], sq2[:],
                                         start=(do == 0), stop=(do == 7))
                    srt2 = wk.tile([128, NT], F32, tag="srt")
                    nc.scalar.activation(srt2[:], ps_s2[:], AF.Sqrt,
                                         bias=eps_s[:, 0:1], scale=1.0 / 1024.0)
                    rs2 = wk.tile([128, NT], F32, tag="rs")
                    nc.vector.reciprocal(rs2[:], srt2[:])
                    for d in range(8):
                        md = wk.tile([128, NT], F32, tag="md")
                        nc.vector.tensor_mul(md[:], xa8[:, d, :], rs2[:])
                        nc.sync.dma_start(mT[:, d, NT * t:NT * t + NT], md[:])
    return nc


# ---------------------------------------------------------------------------
# Launch 2: expert MLP
# ---------------------------------------------------------------------------
def build_mlp_nc(rep=1):
    nc = bass.Bass()
    ms = nc.dram_tensor("msT", [128, 8, T], F16, kind="ExternalInput")
    fcw = nc.dram_tensor("fcw", [128, 8, 4, 128], F16, kind="ExternalInput")
    pjw = nc.dram_tensor("pjw", [128, 4, 8, 128], F16, kind="ExternalInput")
    y = nc.dram_tensor("yT", [128, 8, T], F32, kind="ExternalOutput")

    with tile.TileContext(nc) as tc:
        with (
            tc.tile_pool(name="wp", bufs=1) as wp,
            tc.tile_pool(name="act", bufs=2) as act,
            tc.tile_pool(name="ps", bufs=1, space="PSUM") as psp,
        ):
            fcw_s = wp.tile([128, 8, 4, 128], F16, tag="fcw")
            pjw_s = wp.tile([128, 4, 8, 128], F16, tag="pjw")
            ms_s = wp.tile([128, 8, T], F16, tag="ms")
            y_s = wp.tile([128, 8, T], F32, tag="y")
            nc.sync.dma_start(fcw_s[:], fcw[:])
            nc.sync.dma_start(pjw_s[:], pjw[:])
            nc.sync.dma_start(ms_s[:], ms[:])

            for _ in range(rep):
                for t in range(T // NT):
                    sl = slice(t * NT, (t + 1) * NT)
                    h2 = act.tile([128, 4, NT], F16, tag="h2")
                    for mi in range(4):
                        ph = psp.tile([128, NT], F32, tag="ph", bufs=2)
                        for k in range(8):
                            nc.tensor.matmul(ph[:], fcw_s[:, k, mi, :],
                                             ms_s[:, k, sl],
                                             start=(k == 0), stop=(k == 7))
                        r = act.tile([128, NT], F16, tag="r")
                        nc.scalar.activation(r[:], ph[:], AF.Relu)
                        nc.vector.tensor_mul(h2[:, mi, :], r[:], r[:])
                    for do in range(8):
                        py = psp.tile([128, NT], F32, tag="py", bufs=2)
                        for ki in range(4):
                            nc.tensor.matmul(py[:], pjw_s[:, ki, do, :],
                                             h2[:, ki, :],
                                             start=(ki == 0), stop=(ki == 3))
                        nc.scalar.activation(y_s[:, do, sl], py[:], AF.Copy)
            nc.sync.dma_start(y[:], y_s[:])
    return nc


# ---------------------------------------------------------------------------
# Host-side packing
# ---------------------------------------------------------------------------
def tile_chanmajor(a_T):
    return np.ascontiguousarray(a_T.reshape(8, 128, -1).transpose(1, 0, 2))


def untile_chanmajor(a):
    return np.ascontiguousarray(a.transpose(1, 0, 2)).reshape(1024, -1)


def pack_proj_T(w):
    return np.ascontiguousarray(w.T.reshape(8, 128, 8, 128).transpose(1, 0, 2, 3))


def pack_conv(conv_w):
    tmp = conv_w.reshape(16, 64, 64, 32, 2)
    return np.ascontiguousarray(
        tmp.transpose(4, 2, 0, 3, 1).reshape(128, 16, 32, 64))


def pack_vec(v):
    return np.ascontiguousarray(v.reshape(8, 128).T)


def pack_fcw(fc_w_e):
    return np.ascontiguousarray(
        fc_w_e.reshape(8, 128, 4, 128).transpose(1, 0, 2, 3))


def pack_pjw(proj_w_e):
    return np.ascontiguousarray(
        proj_w_e.reshape(4, 128, 8, 128).transpose(1, 0, 2, 3))


def make_masks():
    head = np.arange(1024) // 64
    ch = head.reshape(8, 128)
    hm = np.zeros((128, 8, 16), np.float32)
    bm = np.zeros((16, 8, 128), np.float32)
    for dd in range(8):
        for p in range(128):
            hm[p, dd, ch[dd, p]] = 1.0
            bm[ch[dd, p], dd, p] = 1.0
    return hm, bm


_CACHE = {}


def _get_ncs():
    if "attn" not in _CACHE:
        _apply_patches()
        _CACHE["attn"] = build_attn_nc()
        _CACHE["mlp"] = build_mlp_nc()
    return _CACHE["attn"], _CACHE["mlp"]


def kernel(x, x0, mu, beta, q_proj_w, conv_w, out_proj_w, fc_w, proj_w,
           attn_scale, mlp_scale, resid_mix, sort_idx):
    from concourse.bass_utils import run_bass_kernel_spmd

    nc1, nc2 = _get_ncs()
    f32 = np.float32
    x = np.asarray(x, f32)
    x0 = np.asarray(x0, f32)
    mu = np.asarray(mu, f32)
    beta = np.asarray(beta, f32)
    q_proj_w = np.asarray(q_proj_w, f32)
    conv_w = np.asarray(conv_w, f32)
    out_proj_w = np.asarray(out_proj_w, f32)
    fc_w = np.asarray(fc_w, f32)
    proj_w = np.asarray(proj_w, f32)
    attn_scale = np.asarray(attn_scale, f32)
    mlp_scale = np.asarray(mlp_scale, f32)
    resid_mix = np.asarray(resid_mix, f32)
    idx = np.asarray(sort_idx).astype(np.int64)

    hm, bm = make_masks()
    common = {
        "qw": pack_proj_T(q_proj_w).astype(ml_dtypes.bfloat16),
        "ow": pack_proj_T(out_proj_w).astype(ml_dtypes.bfloat16),
        "cw": pack_conv(conv_w).astype(ml_dtypes.bfloat16),
        "hmask": hm.astype(ml_dtypes.bfloat16),
        "bmask": bm.astype(ml_dtypes.bfloat16),
        "muc": pack_vec(mu.reshape(-1)),
        "negg": (-np.log1p(np.exp(beta))).reshape(16, 1).astype(f32),
        "rm0": pack_vec(resid_mix[0]),
        "rm1": pack_vec(resid_mix[1]),
        "asc": pack_vec(attn_scale),
        "onesr": np.ones((128, 128), np.float16),
        "epsb": np.full((128, 1), EPS, f32),
    }
    in_maps1 = []
    for c in range(8):
        b, h = c // 2, c % 2
        s0 = h * 2048
        ext = np.zeros((1024, EXT), f32)
        ext0 = np.zeros((1024, EXT), f32)
        ext[:, HALO:] = x[b, s0:s0 + T].T
        ext0[:, HALO:] = x0[b, s0:s0 + T].T
        if s0 > 0:
            ext[:, :HALO] = x[b, s0 - HALO:s0].T
            ext0[:, :HALO] = x0[b, s0 - HALO:s0].T
        in_maps1.append({"xT": tile_chanmajor(ext), "x0T": tile_chanmajor(ext0),
                         **common})

    res1 = run_bass_kernel_spmd(nc1, in_maps1, core_ids=list(range(8)))

    # host routing: token-major xa, expert-sorted m
    xa_tok = np.concatenate(
        [untile_chanmajor(res1.results[c]["xaT"]).T for c in range(8)], axis=0)
    m_chan = np.concatenate(
        [untile_chanmajor(res1.results[c]["mT"]) for c in range(8)], axis=1)
    ms_all = m_chan[:, idx]                          # [1024, 16384] sorted

    in_maps2 = []
    for c in range(8):
        in_maps2.append({
            "msT": tile_chanmajor(ms_all[:, c * T:(c + 1) * T]).astype(np.float16),
            "fcw": pack_fcw(fc_w[c]).astype(np.float16),
            "pjw": pack_pjw(proj_w[c]).astype(np.float16),
        })
    res2 = run_bass_kernel_spmd(nc2, in_maps2, core_ids=list(range(8)))

    y_sorted_tok = np.concatenate(
        [untile_chanmajor(res2.results[c]["yT"]).T for c in range(8)], axis=0)

    out = xa_tok
    out[idx] += mlp_scale[None, :] * y_sorted_tok
    return np.ascontiguousarray(out.reshape(4, 4096, 1024), dtype=f32)
